# revision 15
# baseline (speedup 1.0000x reference)
"""Trainium2 Bass kernel for nn_DenseRNNBase (GRU with dense skip history).

Sharding: data-parallel over batch B=128 across 8 cores. Each core covers 32
batch rows (its 16 primary + the next core's 16, wrapped — a double cover so
the PSUM regions and matmul M-dims are 32-aligned) and writes out only its 16
primary rows. Zero inter-core communication.

Per-core recurrence (step t, j=t%8 block-local):
  gates(t) = h[t-1] @ (W_hh [+W_dense[0] if j>=1]) + sum_{d=1}^{j-1} h[t-1-d] @ W_dense[d]
             + x_t @ W_ih + biases
  r,z = sigmoid(...)   n = tanh(i_n + r*(h_n + b_hh_n))   h[t] = z*h[t-1] + (1-z)*n

Key structure:
- Two PSUM areas: A holds the [32,1536] gate regions of steps 0..3 (at
  partition offset 32*j), B holds steps 4..7. While one half-block computes,
  the other area is refilled for the NEXT half-block: the x-projection
  (float32r, full PE rate) writes its r,z slices directly into PSUM, its
  i_n slice detours through the n columns and is evicted to SBUF (i_n must
  stay separate: n = tanh(i_n + r*s_n)), and ones-row matmuls add the
  biases. Each step's W_hh stream is then the region's last writer (stop).
- Within a block of 8 steps every dense-skip source lies inside the block,
  so an 8-slot ring of transposed hidden states (4 PE transposes per step
  into two alternating PSUM tiles so transposes don't WAR-serialize behind
  the hist copies) feeds all matmuls. Dense-lag terms PACK several target
  steps into one weight stream via the stationary M dim.
- All pack/x-projection/refill work is scheduled into the per-step windows
  where the PE would otherwise idle while the GRU cell's DVE/ACT chain runs,
  keeping the PE busy (HAM stays at full clock).
- The GRU cell runs on DVE/ACT/Pool at the region's partition offset.
"""

import sys

for _p in ("/opt/trn_rl_repo",):
    if _p not in sys.path:
        sys.path.insert(0, _p)

import numpy as np
import ml_dtypes

import concourse.bass as bass
import concourse.bacc as bacc
import concourse.tile as tile
import concourse.mybir as mybir
from concourse import bass_utils

F32 = mybir.dt.float32
F32R = mybir.dt.float32r
BF16 = mybir.dt.bfloat16
AF = mybir.ActivationFunctionType

T, B, I, H, D = 256, 128, 512, 512, 8
G = 3 * H  # 1536
NCORES = 8
BL = 32   # local batch (double cover)
BP = 16   # primary batch rows written out
KI = I // 128
KH = H // 128
NG = G // 512


def build_nc(t_steps=T, reps=1):
    nc = bacc.Bacc("TRN2", target_bir_lowering=False, debug=False)

    xT_d = nc.dram_tensor("xT", (I, t_steps * BL), F32R, kind="ExternalInput")
    h0T_d = nc.dram_tensor("h0T", (H, BL), F32, kind="ExternalInput")
    h0n_d = nc.dram_tensor("h0n", (BL, H), F32, kind="ExternalInput")
    wih_d = nc.dram_tensor("wih", (I, G), F32R, kind="ExternalInput")
    whh_d = nc.dram_tensor("whh", (H, G), BF16, kind="ExternalInput")
    whh0_d = nc.dram_tensor("whh0", (H, G), BF16, kind="ExternalInput")
    wd_d = [nc.dram_tensor(f"wd{d}", (H, G), BF16, kind="ExternalInput")
            for d in range(1, 7)]
    biasn_d = nc.dram_tensor("biasn", (128, H), F32, kind="ExternalInput")
    brz_d = nc.dram_tensor("brz", (1, 2 * H), BF16, kind="ExternalInput")
    bhhn_d = nc.dram_tensor("bhhn", (1, H), BF16, kind="ExternalInput")
    ones_d = nc.dram_tensor("ones", (1, 128), BF16, kind="ExternalInput")
    identf_d = nc.dram_tensor("identf", (128, BL), F32, kind="ExternalInput")
    ys_d = nc.dram_tensor("ys", (t_steps, BP, H), F32, kind="ExternalOutput")

    nblk = t_steps // 8

    with tile.TileContext(nc) as tc:
        with (
            tc.tile_pool(name="wpool", bufs=1) as wpool,
            tc.tile_pool(name="cpool", bufs=1) as cpool,
            tc.tile_pool(name="xt", bufs=2) as xtp,
            tc.tile_pool(name="sbel", bufs=1) as sbp,
            tc.tile_pool(name="hyp", bufs=2) as hyp,
            tc.tile_pool(name="ct", bufs=1) as ct,
            tc.tile_pool(name="pa", bufs=1, space="PSUM") as pap,
            tc.tile_pool(name="pt", bufs=1, space="PSUM") as ptp,
        ):
            # ---- resident weights ----
            wih = wpool.tile([128, KI, G], F32R, tag="wih")
            whh = wpool.tile([128, KH, G], BF16, tag="whh")
            whh0 = wpool.tile([128, KH, G], BF16, tag="whh0")
            wd = [wpool.tile([128, KH, G], BF16, tag=f"wd{d}", name=f"wd{d}")
                  for d in range(6)]
            for k in range(KI):
                nc.sync.dma_start(wih[:, k, :], wih_d[k * 128:(k + 1) * 128, :])
            for k in range(KH):
                nc.sync.dma_start(whh[:, k, :], whh_d[k * 128:(k + 1) * 128, :])
                nc.sync.dma_start(whh0[:, k, :], whh0_d[k * 128:(k + 1) * 128, :])
                for d in range(6):
                    nc.sync.dma_start(wd[d][:, k, :], wd_d[d][k * 128:(k + 1) * 128, :])

            biasn_s = cpool.tile([128, H], F32, tag="biasn")
            nc.sync.dma_start(biasn_s[:], biasn_d[:])
            identf_s = cpool.tile([128, BL], F32, tag="identf")
            nc.sync.dma_start(identf_s[:], identf_d[:])
            brz_f = cpool.tile([1, 2 * H], BF16, tag="brzf")
            nc.sync.dma_start(brz_f[:], brz_d[:])
            bhhn_f = cpool.tile([1, H], BF16, tag="bhhnf")
            nc.sync.dma_start(bhhn_f[:], bhhn_d[:])
            ones_f = cpool.tile([1, 128], BF16, tag="onesf")
            nc.sync.dma_start(ones_f[:], ones_d[:])

            # transposed hidden-state ring: slot s holds hyT of step t%8==s
            hist = cpool.tile([128, KH, 8, BL], BF16, tag="hist")
            h0T_s = cpool.tile([128, KH, BL], F32, tag="h0Ts")
            for k in range(KH):
                nc.sync.dma_start(h0T_s[:, k, :], h0T_d[k * 128:(k + 1) * 128, :])

            # transpose staging: two PSUM tiles so transpose q_{k+1} does not
            # WAR-serialize behind the hist copy of q_k
            ptt_e = ptp.tile([128, 2, BL], F32, tag="pte")
            ptt_o = ptp.tile([128, 2, BL], F32, tag="pto")

            def ptt_slice(k):
                return (ptt_e if k % 2 == 0 else ptt_o)[:, k // 2, :]

            for rep in range(reps):
                for k in range(KH):
                    nc.vector.tensor_copy(hist[:, k, 7, :], h0T_s[:, k, :])
                h_prev = hyp.tile([128, H], F32, tag="hy", name=f"hprev{rep}")
                nc.sync.dma_start(h_prev[96:128, :], h0n_d[:])
                po_prev = 96

                def xt_load(m):
                    """DMA the xT slice for block m."""
                    xt_t = xtp.tile([128, KI, 8 * BL], F32R, tag="xt",
                                    name=f"xt_{rep}_{m}")
                    for k in range(KI):
                        nc.sync.dma_start(
                            xt_t[:, k, :],
                            xT_d[k * 128:(k + 1) * 128, m * 8 * BL:(m + 1) * 8 * BL])
                    return xt_t

                def area_alloc(which, m):
                    return pap.tile([128, G], F32, tag=which,
                                    name=f"{which}_{rep}_{m}")

                def sbn_alloc(g, m):
                    return sbp.tile([128, H], F32, tag=f"sbn{g}",
                                    name=f"sbn{g}_{rep}_{m}")

                def xpiece(area, xt_t, g, n, stop=False):
                    """x-projection (g, n) piece directly into the area."""
                    for k in range(KI):
                        nc.tensor.matmul(
                            area[:, n * 512:(n + 1) * 512],
                            xt_t[:, k, g * 128:(g + 1) * 128],
                            wih[:, k, n * 512:(n + 1) * 512],
                            start=(k == 0), stop=(stop and k == KI - 1),
                            skip_group_check=True)

                def refill_phases(area, xt_t, g, sbn_t):
                    """Refill one area for the next half-block (steps
                    4g..4g+3 of xt_t's block): returns emission phases."""
                    def ph0():  # i_n projection via the n columns + eviction
                        xpiece(area, xt_t, g, 2, stop=True)
                        nc.vector.tensor_add(sbn_t[:], area[:, 1024:1536],
                                             biasn_s[:])

                    def ph1():  # r,z projections (own the r,z columns)
                        xpiece(area, xt_t, g, 0)
                        xpiece(area, xt_t, g, 1)

                    def ph2():  # biases: r,z add; b_hh_n restarts n columns
                        for n in range(2):
                            nc.tensor.matmul(
                                area[:, n * 512:(n + 1) * 512],
                                ones_f[:], brz_f[:, n * 512:(n + 1) * 512],
                                start=False, stop=False, skip_group_check=True)
                        nc.tensor.matmul(
                            area[:, 1024:1536], ones_f[:], bhhn_f[:],
                            start=True, stop=False, skip_group_check=True)

                    return [ph0, ph1, ph2]

                # ---- prologue: load x(0), refill A0 (steps 0-3); B0 gets
                # refilled in windows 0-1 of block 0 like every block's B ----
                xt_cur = xt_load(0)
                areaA = area_alloc("A", 0)
                areaB = None
                sbn = [None, None]
                sbn[0] = sbn_alloc(0, 0)
                for ph in refill_phases(areaA, xt_cur, 0, sbn[0]):
                    ph()

                for m in range(nblk):
                    areas = (areaA, areaB)

                    def pack(d, j1, cnt):
                        """Dense-lag pack: one W_dense[d] weight stream fills
                        regions j1..j1+cnt-1 via stationary-M packing."""
                        s0 = j1 - 1 - d
                        pb = 32 * (j1 % 4)
                        ar = areas[j1 // 4]
                        tps = (0, 96) if pb == 96 else None
                        for k in range(KH):
                            for n in range(NG):
                                nc.tensor.matmul(
                                    ar[pb:pb + 32 * cnt, n * 512:(n + 1) * 512],
                                    hist[:, k, s0:s0 + cnt, :],
                                    wd[d - 1][:, k, n * 512:(n + 1) * 512],
                                    start=False, stop=False,
                                    tile_position=tps,
                                    skip_group_check=True)

                    # pre[j]: emitted before step j's W_hh (executes between
                    # the previous cell's transposes and the W_hh stream).
                    # win[j]: emitted after step j's cell (executes on the PE
                    # while the cell's DVE/ACT chain runs).
                    pre = {2: [lambda: pack(1, 2, 2)],
                           4: [lambda: pack(3, 4, 4), lambda: pack(1, 4, 2)],
                           6: [lambda: pack(1, 6, 2)]}
                    win = {1: [lambda: pack(6, 7, 1), lambda: pack(2, 3, 1)],
                           2: [lambda: pack(5, 6, 2), lambda: pack(2, 4, 2)],
                           # (a 96-row pack at partition 32 is illegal on HW:
                           # partition spans must be 32/64/128-aligned)
                           4: [lambda: pack(4, 5, 1), lambda: pack(4, 6, 2)],
                           5: [lambda: pack(2, 6, 2)]}

                    # refills scheduled into this block's windows:
                    #  B(this block, steps 4-7) was refilled in wins 0-1 of
                    #  this block; A(next block, steps 0-3) in wins 5-7.
                    if m + 1 < nblk:
                        xt_next = xt_load(m + 1)
                        areaA_next = None
                        sbn0_next = None

                        def alloc_refill_A():
                            nonlocal areaA_next, sbn0_next
                            areaA_next = area_alloc("A", m + 1)
                            sbn0_next = sbn_alloc(0, m + 1)
                            return refill_phases(areaA_next, xt_next, 0,
                                                 sbn0_next)

                        phases_holder = []

                        def phA(i):
                            def run():
                                if not phases_holder:
                                    phases_holder.append(alloc_refill_A())
                                phases_holder[0][i]()
                            return run

                        win.setdefault(3, []).append(phA(0))
                        win.setdefault(6, []).append(phA(1))
                        win.setdefault(7, []).append(phA(2))

                    for j in range(8):
                        t = 8 * m + j
                        po = 32 * (j % 4)
                        gi = j // 4
                        area = areas[gi]
                        tp96 = (0, 96) if po == 96 else None

                        # h_prev aligned copy for this step's cell (Pool runs
                        # it during the W_hh stream)
                        s = slice(po, po + 32)
                        h_cur = ct.tile([128, H], F32, tag="hc")
                        nc.gpsimd.tensor_copy(
                            h_cur[s, :], h_prev[po_prev:po_prev + 32, :])

                        for fn in pre.get(j, []):
                            fn()

                        # W_hh (or W_hh + W_dense[0]) term: the region's last
                        # writer; k3 chunks carry stop per column group
                        wsel = whh0 if j >= 1 else whh
                        for k in range(KH):
                            for n in range(NG):
                                nc.tensor.matmul(
                                    area[po:po + 32, n * 512:(n + 1) * 512],
                                    hist[:, k, (t - 1) % 8, :],
                                    wsel[:, k, n * 512:(n + 1) * 512],
                                    start=False, stop=(k == KH - 1),
                                    tile_position=tp96, skip_group_check=True)

                        # ---- GRU cell at partition offset po ----
                        # one fused sigmoid over the adjacent r,z slices: the
                        # PSUM area's accessors serialize, so fewer readers
                        # shorten the serial chain
                        rz = ct.tile([128, 2 * H], F32, tag="rz")
                        nc.scalar.activation(rz[s, :], area[s, 0:1024], AF.Sigmoid)
                        rn = ct.tile([128, H], F32, tag="rn")
                        a_n = ct.tile([128, H], F32, tag="an")
                        nn = ct.tile([128, H], F32, tag="nn")
                        w1 = ct.tile([128, H], F32, tag="w1")  # 1 - z
                        zh = ct.tile([128, H], F32, tag="zh")
                        wn = ct.tile([128, H], F32, tag="wn")
                        # n-chain in pieces (quarters for cols 0:256, a half
                        # for 256:512), DVE emission interleaved so hy of the
                        # first quarter — and with it the next step's W_hh
                        # k0 — lands as early as possible
                        hy = hyp.tile([128, H], F32, tag="hy")

                        def rn_an(lo, hi):
                            cs = slice(lo, hi)
                            nc.vector.tensor_mul(rn[s, cs], rz[s, cs],
                                                 area[s, slice(1024 + lo, 1024 + hi)])
                            nc.vector.tensor_add(a_n[s, cs], rn[s, cs],
                                                 sbn[gi][s, cs])

                        def tanh(lo, hi):
                            cs = slice(lo, hi)
                            nc.scalar.activation(nn[s, cs], a_n[s, cs], AF.Tanh)

                        def wn_hy(lo, hi):
                            cs = slice(lo, hi)
                            nc.vector.tensor_mul(wn[s, cs], w1[s, cs], nn[s, cs])
                            nc.vector.tensor_add(hy[s, cs], zh[s, cs], wn[s, cs])

                        def t_cp(k):
                            pq = ptt_slice(k)
                            nc.tensor.transpose(
                                pq, hy[s, k * 128:(k + 1) * 128],
                                identf_s[s, :], tile_position=(po, 0))
                            nc.scalar.copy(hist[:, k, j, :], pq)

                        rn_an(0, 128)
                        rn_an(128, 256)
                        for half in range(2):
                            cs = slice(half * 256, (half + 1) * 256)
                            zs = slice(512 + half * 256, 512 + (half + 1) * 256)
                            nc.gpsimd.tensor_scalar(
                                w1[s, cs], rz[s, zs], -1.0, 1.0,
                                mybir.AluOpType.mult, mybir.AluOpType.add)
                            nc.gpsimd.tensor_mul(zh[s, cs], rz[s, zs], h_cur[s, cs])
                        tanh(0, 128)
                        tanh(128, 256)
                        wn_hy(0, 128)
                        t_cp(0)
                        rn_an(256, 512)
                        tanh(256, 512)
                        wn_hy(128, 256)
                        t_cp(1)
                        wn_hy(256, 512)
                        t_cp(2)
                        t_cp(3)

                        nc.sync.dma_start(ys_d[t], hy[po:po + BP, :])

                        # B refill for THIS block's steps 4-7 in wins 0-1
                        if j == 0:
                            areaB = area_alloc("B", m)
                            sbn[1] = sbn_alloc(1, m)
                            phB = refill_phases(areaB, xt_cur, 1, sbn[1])
                            phB[0]()
                            phB[1]()
                            areas = (areaA, areaB)
                        elif j == 1:
                            phB[2]()

                        for fn in win.get(j, []):
                            if fn is not None:
                                fn()

                        h_prev = hy
                        po_prev = po

                    if m + 1 < nblk:
                        areaA = phases_holder and areaA_next or areaA
                        sbn[0] = sbn0_next
                        xt_cur = xt_next

    nc.compile()
    return nc


def round_f32r(a):
    """Round fp32 to the PE's float32r (tf32-like 10-bit mantissa)."""
    a = np.ascontiguousarray(a, dtype=np.float32)
    v = a.view(np.uint32).copy()
    v += 0x1000 + ((v >> 13) & 1)
    v &= 0xFFFFE000
    return v.view(np.float32)


def host_prep(x, h0, W_ih, W_hh, b_ih, b_hh, W_dense, t_steps=T):
    """Build per-core in_maps."""
    bf = ml_dtypes.bfloat16
    whh0 = (W_hh + W_dense[0]).astype(bf)
    whh_b = W_hh.astype(bf)
    wd_b = [W_dense[d].astype(bf) for d in range(1, 7)]
    biasn = np.broadcast_to(b_ih[2 * H:][None, :], (128, H)).astype(np.float32).copy()
    brz = (b_ih + b_hh)[:2 * H].reshape(1, 2 * H).astype(bf)
    bhhn = b_hh[2 * H:].reshape(1, H).astype(bf)
    ones = np.ones((1, 128), bf)
    ident = np.tile(np.eye(BL, dtype=np.float32), (4, 1))
    wih_f = round_f32r(W_ih)

    in_maps = []
    for c in range(NCORES):
        idx = (16 * c + np.arange(BL)) % B
        xc = x[:t_steps, idx, :]
        xT = np.ascontiguousarray(xc.transpose(2, 0, 1).reshape(I, t_steps * BL))
        m = {
            "xT": round_f32r(xT),
            "h0T": np.ascontiguousarray(h0[idx].T).astype(np.float32),
            "h0n": h0[idx].astype(np.float32),
            "wih": wih_f, "whh": whh_b, "whh0": whh0,
            "biasn": biasn, "brz": brz, "bhhn": bhhn, "ones": ones,
            "identf": ident,
        }
        for d in range(6):
            m[f"wd{d + 1}"] = wd_b[d]
        in_maps.append(m)
    return in_maps


_NC_CACHE = {}


def _get_nc(t_steps=T):
    if t_steps not in _NC_CACHE:
        _NC_CACHE[t_steps] = build_nc(t_steps)
    return _NC_CACHE[t_steps]


def kernel(x, h0, W_ih, W_hh, b_ih, b_hh, W_dense):
    x = np.asarray(x, dtype=np.float32)
    h0 = np.asarray(h0, dtype=np.float32)
    nc = _get_nc(T)
    in_maps = host_prep(x, h0, np.asarray(W_ih), np.asarray(W_hh),
                        np.asarray(b_ih), np.asarray(b_hh), np.asarray(W_dense))
    res = bass_utils.run_bass_kernel_spmd(nc, in_maps, core_ids=list(range(NCORES)))
    ys = np.empty((T, B, H), dtype=np.float32)
    for c in range(NCORES):
        ys[:, 16 * c:16 * c + BP, :] = res.results[c]["ys"]
    return ys


# revision 17
# speedup vs baseline: 38.5308x; 38.5308x over previous
"""Trainium2 Bass kernel for nn_DenseRNNBase (GRU with dense skip history).

Sharding: data-parallel over batch B=128 across 8 cores. Each core covers 32
batch rows (its 16 primary + the next core's 16, wrapped — a double cover so
the PSUM regions and matmul M-dims are 32-aligned) and writes out only its 16
primary rows. Zero inter-core communication.

Per-core recurrence (step t, j=t%8 block-local):
  gates(t) = h[t-1] @ (W_hh [+W_dense[0] if j>=1]) + sum_{d=1}^{j-1} h[t-1-d] @ W_dense[d]
             + x_t @ W_ih + biases
  r,z = sigmoid(...)   n = tanh(i_n + r*(h_n + b_hh_n))   h[t] = z*h[t-1] + (1-z)*n

Key structure:
- Two PSUM areas: A holds the [32,1536] gate regions of steps 0..3 (at
  partition offset 32*j), B holds steps 4..7. While one half-block computes,
  the other area is refilled for the NEXT half-block: the x-projection
  (float32r, full PE rate) writes its r,z slices directly into PSUM, its
  i_n slice detours through the n columns and is evicted to SBUF (i_n must
  stay separate: n = tanh(i_n + r*s_n)), and ones-row matmuls add the
  biases. Each step's W_hh stream is then the region's last writer (stop).
- Within a block of 8 steps every dense-skip source lies inside the block,
  so an 8-slot ring of transposed hidden states (4 PE transposes per step
  into two alternating PSUM tiles so transposes don't WAR-serialize behind
  the hist copies) feeds all matmuls. Dense-lag terms PACK several target
  steps into one weight stream via the stationary M dim.
- All pack/x-projection/refill work is scheduled into the per-step windows
  where the PE would otherwise idle while the GRU cell's DVE/ACT chain runs,
  keeping the PE busy (HAM stays at full clock).
- The GRU cell runs on DVE/ACT/Pool at the region's partition offset.
"""

import sys

for _p in ("/opt/trn_rl_repo",):
    if _p not in sys.path:
        sys.path.insert(0, _p)

import numpy as np
import ml_dtypes

import concourse.bass as bass
import concourse.bacc as bacc
import concourse.tile as tile
import concourse.mybir as mybir
from concourse import bass_utils

F32 = mybir.dt.float32
F32R = mybir.dt.float32r
BF16 = mybir.dt.bfloat16
AF = mybir.ActivationFunctionType

T, B, I, H, D = 256, 128, 512, 512, 8
G = 3 * H  # 1536
NCORES = 8
BL = 32   # local batch (double cover)
BP = 16   # primary batch rows written out
KI = I // 128
KH = H // 128
NG = G // 512


def build_nc(t_steps=T, reps=1):
    nc = bacc.Bacc("TRN2", target_bir_lowering=False, debug=False)

    xT_d = nc.dram_tensor("xT", (I, t_steps * BL), F32R, kind="ExternalInput")
    h0T_d = nc.dram_tensor("h0T", (H, BL), F32, kind="ExternalInput")
    h0n_d = nc.dram_tensor("h0n", (BL, H), F32, kind="ExternalInput")
    wih_d = nc.dram_tensor("wih", (I, G), F32R, kind="ExternalInput")
    whh_d = nc.dram_tensor("whh", (H, G), BF16, kind="ExternalInput")
    whh0_d = nc.dram_tensor("whh0", (H, G), BF16, kind="ExternalInput")
    wd_d = [nc.dram_tensor(f"wd{d}", (H, G), BF16, kind="ExternalInput")
            for d in range(1, 7)]
    biasn_d = nc.dram_tensor("biasn", (128, H), F32, kind="ExternalInput")
    brz_d = nc.dram_tensor("brz", (1, 2 * H), BF16, kind="ExternalInput")
    bhhn_d = nc.dram_tensor("bhhn", (1, H), BF16, kind="ExternalInput")
    ones_d = nc.dram_tensor("ones", (1, 128), BF16, kind="ExternalInput")
    identf_d = nc.dram_tensor("identf", (128, BL), F32, kind="ExternalInput")
    ys_d = nc.dram_tensor("ys", (t_steps, BP, H), F32, kind="ExternalOutput")

    nblk = t_steps // 8

    with tile.TileContext(nc) as tc:
        with (
            tc.tile_pool(name="wpool", bufs=1) as wpool,
            tc.tile_pool(name="cpool", bufs=1) as cpool,
            tc.tile_pool(name="xt", bufs=2) as xtp,
            tc.tile_pool(name="sbel", bufs=1) as sbp,
            tc.tile_pool(name="hyp", bufs=2) as hyp,
            tc.tile_pool(name="ct", bufs=1) as ct,
            tc.tile_pool(name="pa", bufs=1, space="PSUM") as pap,
            tc.tile_pool(name="pt", bufs=1, space="PSUM") as ptp,
        ):
            # ---- resident weights ----
            wih = wpool.tile([128, KI, G], F32R, tag="wih")
            whh = wpool.tile([128, KH, G], BF16, tag="whh")
            whh0 = wpool.tile([128, KH, G], BF16, tag="whh0")
            wd = [wpool.tile([128, KH, G], BF16, tag=f"wd{d}", name=f"wd{d}")
                  for d in range(6)]
            for k in range(KI):
                nc.sync.dma_start(wih[:, k, :], wih_d[k * 128:(k + 1) * 128, :])
            for k in range(KH):
                nc.sync.dma_start(whh[:, k, :], whh_d[k * 128:(k + 1) * 128, :])
                nc.sync.dma_start(whh0[:, k, :], whh0_d[k * 128:(k + 1) * 128, :])
                for d in range(6):
                    nc.sync.dma_start(wd[d][:, k, :], wd_d[d][k * 128:(k + 1) * 128, :])

            biasn_s = cpool.tile([128, H], F32, tag="biasn")
            nc.sync.dma_start(biasn_s[:], biasn_d[:])
            identf_s = cpool.tile([128, BL], F32, tag="identf")
            nc.sync.dma_start(identf_s[:], identf_d[:])
            brz_f = cpool.tile([1, 2 * H], BF16, tag="brzf")
            nc.sync.dma_start(brz_f[:], brz_d[:])
            bhhn_f = cpool.tile([1, H], BF16, tag="bhhnf")
            nc.sync.dma_start(bhhn_f[:], bhhn_d[:])
            ones_f = cpool.tile([1, 128], BF16, tag="onesf")
            nc.sync.dma_start(ones_f[:], ones_d[:])

            # transposed hidden-state ring: slot s holds hyT of step t%8==s
            hist = cpool.tile([128, KH, 8, BL], BF16, tag="hist")
            h0T_s = cpool.tile([128, KH, BL], F32, tag="h0Ts")
            for k in range(KH):
                nc.sync.dma_start(h0T_s[:, k, :], h0T_d[k * 128:(k + 1) * 128, :])

            # transpose staging: two PSUM tiles so transpose q_{k+1} does not
            # WAR-serialize behind the hist copy of q_k
            ptt_e = ptp.tile([128, 2, BL], F32, tag="pte")
            ptt_o = ptp.tile([128, 2, BL], F32, tag="pto")

            def ptt_slice(k):
                return (ptt_e if k % 2 == 0 else ptt_o)[:, k // 2, :]

            for rep in range(reps):
                for k in range(KH):
                    nc.vector.tensor_copy(hist[:, k, 7, :], h0T_s[:, k, :])
                h_prev = hyp.tile([128, H], F32, tag="hy", name=f"hprev{rep}")
                nc.sync.dma_start(h_prev[96:128, :], h0n_d[:])
                po_prev = 96

                def xt_load(m):
                    """DMA the xT slice for block m."""
                    xt_t = xtp.tile([128, KI, 8 * BL], F32R, tag="xt",
                                    name=f"xt_{rep}_{m}")
                    for k in range(KI):
                        nc.sync.dma_start(
                            xt_t[:, k, :],
                            xT_d[k * 128:(k + 1) * 128, m * 8 * BL:(m + 1) * 8 * BL])
                    return xt_t

                def area_alloc(which, m):
                    return pap.tile([128, G], F32, tag=which,
                                    name=f"{which}_{rep}_{m}")

                def sbn_alloc(g, m):
                    return sbp.tile([128, H], F32, tag=f"sbn{g}",
                                    name=f"sbn{g}_{rep}_{m}")

                def xpiece(area, xt_t, g, n, stop=False):
                    """x-projection (g, n) piece directly into the area."""
                    for k in range(KI):
                        nc.tensor.matmul(
                            area[:, n * 512:(n + 1) * 512],
                            xt_t[:, k, g * 128:(g + 1) * 128],
                            wih[:, k, n * 512:(n + 1) * 512],
                            start=(k == 0), stop=(stop and k == KI - 1),
                            skip_group_check=True)

                def refill_phases(area, xt_t, g, sbn_t):
                    """Refill one area for the next half-block (steps
                    4g..4g+3 of xt_t's block): returns emission phases."""
                    def ph0():  # i_n projection via the n columns + eviction
                        xpiece(area, xt_t, g, 2, stop=True)
                        nc.vector.tensor_add(sbn_t[:], area[:, 1024:1536],
                                             biasn_s[:])

                    def ph1():  # r,z projections (own the r,z columns)
                        xpiece(area, xt_t, g, 0)
                        xpiece(area, xt_t, g, 1)

                    def ph2():  # biases: r,z add; b_hh_n restarts n columns
                        for n in range(2):
                            nc.tensor.matmul(
                                area[:, n * 512:(n + 1) * 512],
                                ones_f[:], brz_f[:, n * 512:(n + 1) * 512],
                                start=False, stop=False, skip_group_check=True)
                        nc.tensor.matmul(
                            area[:, 1024:1536], ones_f[:], bhhn_f[:],
                            start=True, stop=False, skip_group_check=True)

                    return [ph0, ph1, ph2]

                # ---- prologue: load x(0), refill A0 (steps 0-3); B0 gets
                # refilled in windows 0-1 of block 0 like every block's B ----
                xt_cur = xt_load(0)
                areaA = area_alloc("A", 0)
                areaB = None
                sbn = [None, None]
                sbn[0] = sbn_alloc(0, 0)
                for ph in refill_phases(areaA, xt_cur, 0, sbn[0]):
                    ph()

                for m in range(nblk):
                    areas = (areaA, areaB)

                    def pack(d, j1, cnt):
                        """Dense-lag pack: one W_dense[d] weight stream fills
                        regions j1..j1+cnt-1 via stationary-M packing."""
                        s0 = j1 - 1 - d
                        pb = 32 * (j1 % 4)
                        ar = areas[j1 // 4]
                        tps = (0, 96) if pb == 96 else None
                        for k in range(KH):
                            for n in range(NG):
                                nc.tensor.matmul(
                                    ar[pb:pb + 32 * cnt, n * 512:(n + 1) * 512],
                                    hist[:, k, s0:s0 + cnt, :],
                                    wd[d - 1][:, k, n * 512:(n + 1) * 512],
                                    start=False, stop=False,
                                    tile_position=tps,
                                    skip_group_check=True)

                    # pre[j]: emitted before step j's W_hh (executes between
                    # the previous cell's transposes and the W_hh stream).
                    # win[j]: emitted after step j's cell (executes on the PE
                    # while the cell's DVE/ACT chain runs).
                    # Window-fill rules: work emitted in win[j] executes while
                    # cell j's DVE/ACT chain runs, but only if it is NOT an
                    # accessor of the area cell j is reading (PSUM-tile
                    # accessors serialize), and its hist sources exist
                    # (source h[i] is ready only after cell i). A pack
                    # covering region r must be emitted before step r's W_hh
                    # (the region's stop). Partition spans must be 32/64/128-
                    # aligned, so e.g. d4 -> (5) + (6,7) streams.
                    pre = {2: [lambda: pack(1, 2, 2)],
                           4: [lambda: pack(3, 4, 4), lambda: pack(1, 4, 2)],
                           6: [lambda: pack(1, 6, 2)]}
                    win = {1: [lambda: pack(6, 7, 1), lambda: pack(2, 3, 1)],
                           2: [lambda: pack(5, 6, 2), lambda: pack(2, 4, 2)],
                           3: [lambda: pack(4, 5, 1), lambda: pack(4, 6, 2)],
                           5: [lambda: pack(2, 6, 2)]}

                    # refills scheduled into this block's windows:
                    #  B(this block, steps 4-7) was refilled in wins 0-1 of
                    #  this block; A(next block, steps 0-3) in wins 5-7.
                    if m + 1 < nblk:
                        xt_next = xt_load(m + 1)
                        areaA_next = None
                        sbn0_next = None

                        def alloc_refill_A():
                            nonlocal areaA_next, sbn0_next
                            areaA_next = area_alloc("A", m + 1)
                            sbn0_next = sbn_alloc(0, m + 1)
                            return refill_phases(areaA_next, xt_next, 0,
                                                 sbn0_next)

                        phases_holder = []

                        def phA(i):
                            def run():
                                if not phases_holder:
                                    phases_holder.append(alloc_refill_A())
                                phases_holder[0][i]()
                            return run

                        # A'-refill is an A-area accessor: from win4 on it no
                        # longer trails cell 3's A reads
                        win.setdefault(4, []).append(phA(0))
                        win.setdefault(5, []).append(phA(1))
                        win.setdefault(6, []).append(phA(2))

                    for j in range(8):
                        t = 8 * m + j
                        po = 32 * (j % 4)
                        gi = j // 4
                        area = areas[gi]
                        tp96 = (0, 96) if po == 96 else None

                        # h_prev aligned copy for this step's cell (Pool runs
                        # it during the W_hh stream)
                        s = slice(po, po + 32)
                        h_cur = ct.tile([128, H], F32, tag="hc")
                        nc.gpsimd.tensor_copy(
                            h_cur[s, :], h_prev[po_prev:po_prev + 32, :])

                        for fn in pre.get(j, []):
                            fn()

                        # W_hh (or W_hh + W_dense[0]) term: the region's last
                        # writer; k3 chunks carry stop per column group
                        wsel = whh0 if j >= 1 else whh
                        for k in range(KH):
                            for n in range(NG):
                                nc.tensor.matmul(
                                    area[po:po + 32, n * 512:(n + 1) * 512],
                                    hist[:, k, (t - 1) % 8, :],
                                    wsel[:, k, n * 512:(n + 1) * 512],
                                    start=False, stop=(k == KH - 1),
                                    tile_position=tp96, skip_group_check=True)

                        # ---- GRU cell at partition offset po ----
                        # one fused sigmoid over the adjacent r,z slices: the
                        # PSUM area's accessors serialize, so fewer readers
                        # shorten the serial chain
                        rz = ct.tile([128, 2 * H], F32, tag="rz")
                        nc.scalar.activation(rz[s, :], area[s, 0:1024], AF.Sigmoid)
                        rn = ct.tile([128, H], F32, tag="rn")
                        a_n = ct.tile([128, H], F32, tag="an")
                        nn = ct.tile([128, H], F32, tag="nn")
                        w1 = ct.tile([128, H], F32, tag="w1")  # 1 - z
                        zh = ct.tile([128, H], F32, tag="zh")
                        wn = ct.tile([128, H], F32, tag="wn")
                        # n-chain in pieces (quarters for cols 0:256, a half
                        # for 256:512), DVE emission interleaved so hy of the
                        # first quarter — and with it the next step's W_hh
                        # k0 — lands as early as possible
                        hy = hyp.tile([128, H], F32, tag="hy")

                        def rn_an(lo, hi):
                            cs = slice(lo, hi)
                            nc.vector.tensor_mul(rn[s, cs], rz[s, cs],
                                                 area[s, slice(1024 + lo, 1024 + hi)])
                            nc.vector.tensor_add(a_n[s, cs], rn[s, cs],
                                                 sbn[gi][s, cs])

                        def tanh(lo, hi):
                            cs = slice(lo, hi)
                            nc.scalar.activation(nn[s, cs], a_n[s, cs], AF.Tanh)

                        def wn_hy(lo, hi):
                            cs = slice(lo, hi)
                            nc.vector.tensor_mul(wn[s, cs], w1[s, cs], nn[s, cs])
                            nc.vector.tensor_add(hy[s, cs], zh[s, cs], wn[s, cs])

                        def t_cp(k):
                            pq = ptt_slice(k)
                            nc.tensor.transpose(
                                pq, hy[s, k * 128:(k + 1) * 128],
                                identf_s[s, :], tile_position=(po, 0))
                            nc.scalar.copy(hist[:, k, j, :], pq)

                        rn_an(0, 128)
                        rn_an(128, 256)
                        for half in range(2):
                            cs = slice(half * 256, (half + 1) * 256)
                            zs = slice(512 + half * 256, 512 + (half + 1) * 256)
                            nc.gpsimd.tensor_scalar(
                                w1[s, cs], rz[s, zs], -1.0, 1.0,
                                mybir.AluOpType.mult, mybir.AluOpType.add)
                            nc.gpsimd.tensor_mul(zh[s, cs], rz[s, zs], h_cur[s, cs])
                        tanh(0, 128)
                        tanh(128, 256)
                        wn_hy(0, 128)
                        t_cp(0)
                        rn_an(256, 512)
                        tanh(256, 512)
                        wn_hy(128, 256)
                        t_cp(1)
                        wn_hy(256, 512)
                        t_cp(2)
                        t_cp(3)

                        nc.sync.dma_start(ys_d[t], hy[po:po + BP, :])

                        # B refill for THIS block's steps 4-7 in wins 0-1
                        if j == 0:
                            areaB = area_alloc("B", m)
                            sbn[1] = sbn_alloc(1, m)
                            phB = refill_phases(areaB, xt_cur, 1, sbn[1])
                            phB[0]()
                            phB[1]()
                            areas = (areaA, areaB)
                        elif j == 1:
                            phB[2]()

                        for fn in win.get(j, []):
                            if fn is not None:
                                fn()

                        h_prev = hy
                        po_prev = po

                    if m + 1 < nblk:
                        areaA = phases_holder and areaA_next or areaA
                        sbn[0] = sbn0_next
                        xt_cur = xt_next

    nc.compile()
    return nc


def round_f32r(a):
    """Round fp32 to the PE's float32r (tf32-like 10-bit mantissa)."""
    a = np.ascontiguousarray(a, dtype=np.float32)
    v = a.view(np.uint32).copy()
    v += 0x1000 + ((v >> 13) & 1)
    v &= 0xFFFFE000
    return v.view(np.float32)


def host_prep(x, h0, W_ih, W_hh, b_ih, b_hh, W_dense, t_steps=T):
    """Build per-core in_maps."""
    bf = ml_dtypes.bfloat16
    whh0 = (W_hh + W_dense[0]).astype(bf)
    whh_b = W_hh.astype(bf)
    wd_b = [W_dense[d].astype(bf) for d in range(1, 7)]
    biasn = np.broadcast_to(b_ih[2 * H:][None, :], (128, H)).astype(np.float32).copy()
    brz = (b_ih + b_hh)[:2 * H].reshape(1, 2 * H).astype(bf)
    bhhn = b_hh[2 * H:].reshape(1, H).astype(bf)
    ones = np.ones((1, 128), bf)
    ident = np.tile(np.eye(BL, dtype=np.float32), (4, 1))
    wih_f = round_f32r(W_ih)

    in_maps = []
    for c in range(NCORES):
        idx = (16 * c + np.arange(BL)) % B
        xc = x[:t_steps, idx, :]
        xT = np.ascontiguousarray(xc.transpose(2, 0, 1).reshape(I, t_steps * BL))
        m = {
            "xT": round_f32r(xT),
            "h0T": np.ascontiguousarray(h0[idx].T).astype(np.float32),
            "h0n": h0[idx].astype(np.float32),
            "wih": wih_f, "whh": whh_b, "whh0": whh0,
            "biasn": biasn, "brz": brz, "bhhn": bhhn, "ones": ones,
            "identf": ident,
        }
        for d in range(6):
            m[f"wd{d + 1}"] = wd_b[d]
        in_maps.append(m)
    return in_maps


_NC_CACHE = {}


def _get_nc(t_steps=T):
    if t_steps not in _NC_CACHE:
        _NC_CACHE[t_steps] = build_nc(t_steps)
    return _NC_CACHE[t_steps]


def kernel(x, h0, W_ih, W_hh, b_ih, b_hh, W_dense):
    x = np.asarray(x, dtype=np.float32)
    h0 = np.asarray(h0, dtype=np.float32)
    nc = _get_nc(T)
    in_maps = host_prep(x, h0, np.asarray(W_ih), np.asarray(W_hh),
                        np.asarray(b_ih), np.asarray(b_hh), np.asarray(W_dense))
    res = bass_utils.run_bass_kernel_spmd(nc, in_maps, core_ids=list(range(NCORES)))
    ys = np.empty((T, B, H), dtype=np.float32)
    for c in range(NCORES):
        ys[:, 16 * c:16 * c + BP, :] = res.results[c]["ys"]
    return ys
